# revision 13
# baseline (speedup 1.0000x reference)
"""Trainium2 Bass kernel for the DCM sparse-attention problem.

Math restructure: with t-hat/v-hat the row-normalized features and
S[(a,t),(b,v)] = <t-hat[a,t], v-hat[b,v]> the raw cosine logits, every
softmax-weighted aggregation in the reference collapses onto S:

  t2v[a,b,t] = sum_v vps1 * S            (free-dim group reduce)
  v2t[a,b,v] = sum_t tps1 * S            (mask-folded indicator matmul)
  out[a,b]   = sum_t tps2[t] sum_v vps2[v] S[t,v]

so the [A,B,T,D] intermediates never exist. Each of the 8 cores handles
8 of the 64 text rows (A-sharded, video replicated).

Dataflow (v4): video DMAs first and hogs the DMA engines, because the
critical chain is video -> squares -> column-sum matmul -> sqrt ->
reciprocal -> partition-broadcast -> S-hat; that norm chain runs in
384-wide halves to pipeline ACT/DVE/GpSimd.  The S matmul uses the RAW
bf16 features (text arrives later, overlapped), then one fused
scalar_tensor_tensor folds both norms: S-hat = (psum * r_t) * rv,
kept fp32 (bf16 S-hat costs ~20% extra error).  The exp/weight stages
also run in halves so the Scalar engine's unaccelerated ~1ns/elem
activations never gate the DVE.  Activations are ordered SQRT*2 then
EXP-only (the ACT engine reloads its 1.3us table on every function
switch).  Indicator matmuls emit an 8-row layout so the final divides
and the single output DMA are minimal.
"""

import sys

sys.path.insert(0, "/opt/trn_rl_repo")

import ml_dtypes
import numpy as np

import concourse.bass as bass
import concourse.bacc as bacc
import concourse.tile as tile
from concourse import mybir
from concourse.bass_utils import run_bass_kernel_spmd

TAU = 100.0
A, T, B, V, D = 64, 32, 64, 12, 512
NCORES = 8
AL = A // NCORES          # a's per core = 8
AT = AL * T               # (a,t) rows per core = 256
BV = B * V                # (b,v) cols = 768
NMT = AT // 128           # M-tiles over (a,t) = 2
NKT = D // 128            # K-tiles over d = 4
APB = 128 // T            # a's per M-tile = 4
F32 = mybir.dt.float32
BF16 = mybir.dt.bfloat16
EXP = mybir.ActivationFunctionType.Exp
SQRT = mybir.ActivationFunctionType.Sqrt
COPY = mybir.ActivationFunctionType.Copy
MUL = mybir.AluOpType.mult
X = mybir.AxisListType.X
NSL = [(0, 512), (512, 768)]                   # bank-aligned slices of 768
NSL3 = [(0, 512), (512, 1024), (1024, 1536)]   # ... of 1536
HALF = [(0, 384), (384, 768)]                  # group-aligned halves
WSL = [(0, 384), (384, 512), (512, 768)]       # bank-safe W4 chunks


def _build_program():
    nc = bacc.Bacc("TRN2", target_bir_lowering=False)

    # all three feature blobs are host-prearranged so each is one
    # contiguous [128, x] DMA (2-3KB per partition row)
    vT01_d = nc.declare_dram_parameter("vT01", [128, 2 * BV], BF16,
                                       isOutput=False)
    vT23_d = nc.declare_dram_parameter("vT23", [128, 2 * BV], BF16,
                                       isOutput=False)
    tTa_d = nc.declare_dram_parameter("tTa", [128, 4 * AT], BF16,
                                      isOutput=False)
    # cb16: [ind8m (16) | ind8 (16) | onesc (1)]; cbW: broadcast indicator;
    # cb32: [tau_m M0 | tau_m M1 | ident]
    cb16_d = nc.declare_dram_parameter("cb16", [128, 33], BF16, isOutput=False)
    cbW_d = nc.declare_dram_parameter("cbW", [AL, 2 * 128], BF16,
                                      isOutput=False)
    cb32_d = nc.declare_dram_parameter("cb32", [128, 3], F32, isOutput=False)
    out_d = nc.declare_dram_parameter("out", [AL, B], F32, isOutput=True)

    with tile.TileContext(nc) as tc:
        with (
            tc.tile_pool(name="consts", bufs=1) as consts,
            tc.tile_pool(name="inputs", bufs=1) as inputs,
            tc.tile_pool(name="sq", bufs=1) as sqp,
            tc.tile_pool(name="big", bufs=1) as bigp,
            tc.tile_pool(name="smalls", bufs=1) as smalls,
            tc.tile_pool(name="psA", bufs=2, space="PSUM") as psA,
            tc.tile_pool(name="psB", bufs=1, space="PSUM") as psB,
        ):
            # ---- input DMAs, all on the sync queue so the descriptor
            # order enforces video-first bandwidth priority ----
            vh = [inputs.tile([128, 2 * BV], BF16, name=f"vh{h}")
                  for h in range(2)]
            tTa = inputs.tile([128, 1024], BF16)
            cb32 = consts.tile([128, 3], F32)
            cb16 = consts.tile([128, 33], BF16)
            cbW = consts.tile([AL, 2 * 128], BF16)
            nc.sync.dma_start(out=vh[0], in_=vT01_d[:, :])
            nc.sync.dma_start(out=vh[1], in_=vT23_d[:, :])
            nc.sync.dma_start(out=tTa, in_=tTa_d[:, :])
            nc.sync.dma_start(out=cb32, in_=cb32_d[:, :])
            nc.sync.dma_start(out=cb16, in_=cb16_d[:, :])
            nc.sync.dma_start(out=cbW, in_=cbW_d[:, :])
            ident = cb32[0:1, 2:3]
            ind8m = [cb16[:, 8 * i:8 * (i + 1)] for i in range(NMT)]
            ind8 = [cb16[:, 16 + 8 * i:16 + 8 * (i + 1)] for i in range(NMT)]
            onesc = cb16[:, 32:33]

            def vt(k):          # k-tile view of the raw video features
                return vh[k // 2][:, BV * (k % 2):BV * (k % 2 + 1)]

            def tt(k, i):       # stationary slice for M-tile i, k-tile k
                return tTa[:, 256 * k + 128 * i:256 * k + 128 * (i + 1)]

            # ---- video norm chain first (it gates everything): squares
            # on the DVE (bf16 2x), ones-matmul column sums on the PE ----
            ps_ssv = psB.tile([1, BV], F32, tag="v")
            sqv = [sqp.tile([128, 2 * BV], BF16, name=f"sqv{h}")
                   for h in range(2)]
            for h in range(2):
                nc.vector.tensor_tensor(sqv[h], vh[h], vh[h], op=MUL)
                for k2 in range(2):
                    for lo, hi in NSL:
                        nc.tensor.matmul(
                            ps_ssv[:, lo:hi], onesc,
                            sqv[h][:, BV * k2 + lo:BV * k2 + hi],
                            start=(h == 0 and k2 == 0),
                            stop=(h == 1 and k2 == 1))
            ps_sst = psB.tile([1, AT], F32, tag="j")
            sqt = sqp.tile([128, 1024], BF16)
            nc.vector.tensor_tensor(sqt, tTa, tTa, op=MUL)
            for k in range(NKT):
                nc.tensor.matmul(ps_sst, onesc, sqt[:, 256 * k:256 * (k + 1)],
                                 start=(k == 0), stop=(k == NKT - 1))

            # both SQRTs back-to-back, then only EXP from here on: the ACT
            # engine reloads its table on EVERY function switch (~1.3us)
            nt_row = smalls.tile([1, AT], F32)
            nv_row = smalls.tile([1, BV], F32)
            for lo, hi in HALF:
                nc.scalar.activation(nv_row[:, lo:hi], ps_ssv[:, lo:hi], SQRT)
            nc.scalar.activation(nt_row, ps_sst, SQRT)

            # r_v: reciprocal + partition broadcast, in halves so the DVE
            # and GpSimd pipeline
            rv_row = smalls.tile([1, BV], F32)
            rv_bc = bigp.tile([128, BV], F32)
            for lo, hi in HALF:
                nc.vector.reciprocal_approx_fast(rv_row[:, lo:hi],
                                                 nv_row[:, lo:hi])
                nc.gpsimd.partition_broadcast(rv_bc[:, lo:hi],
                                              rv_row[:, lo:hi], channels=128)

            r_t = [smalls.tile([128, 1], F32, name=f"r_t{i}")
                   for i in range(NMT)]
            for i in range(NMT):
                ps_tr = psB.tile([128, 1], F32, tag="j", name=f"ps_tr{i}")
                nc.tensor.transpose(ps_tr, nt_row[:, 128 * i:128 * (i + 1)],
                                    ident)
                nc.vector.reciprocal_approx_fast(r_t[i], ps_tr)

            # ---- S matmuls on the raw features ----
            ps_s = [psA.tile([128, BV], F32, tag="s", name=f"ps_s{i}")
                    for i in range(NMT)]
            for i in range(NMT):
                for lo, hi in NSL:
                    for k in range(NKT):
                        nc.tensor.matmul(
                            ps_s[i][:, lo:hi], tt(k, i), vt(k)[:, lo:hi],
                            start=(k == 0), stop=(k == NKT - 1))

            # ---- per-M-tile softmax prep in halves: one fused STT folds
            # both norms into the PSUM read (S-hat = psum * r_t * rv, fp32),
            # the EXP rides the ACT with the tau*m per-partition scale, and
            # ES = S-hat * E closes the pair.  spb is a bf16 shadow of
            # S-hat so the later w4s weighting can run in the DVE 2x mode.
            big = [bigp.tile([128, 2 * BV], BF16, name=f"big{i}")
                   for i in range(NMT)]
            sp = [bigp.tile([128, BV], F32, name=f"sp{i}") for i in range(NMT)]
            spb = [bigp.tile([128, BV], BF16, name=f"spb{i}")
                   for i in range(NMT)]
            rhs_f = [smalls.tile([128, 128], BF16, name=f"rhs_f{i}")
                     for i in range(NMT)]
            red = [smalls.tile([128, 128], F32, name=f"red{i}")
                   for i in range(NMT)]
            for i in range(NMT):
                for lo, hi in HALF:
                    nc.vector.scalar_tensor_tensor(
                        sp[i][:, lo:hi], ps_s[i][:, lo:hi], r_t[i],
                        rv_bc[:, lo:hi], op0=MUL, op1=MUL)
                    nc.scalar.activation(big[i][:, BV + lo:BV + hi],
                                         sp[i][:, lo:hi], EXP,
                                         scale=cb32[:, i:i + 1])
                    nc.vector.tensor_tensor(big[i][:, lo:hi], sp[i][:, lo:hi],
                                            big[i][:, BV + lo:BV + hi],
                                            op=MUL)
                nc.scalar.activation(spb[i], sp[i], COPY)

            # ---- v2t: mask-folded indicator matmul over t -> [8, 2*BV] ----
            ps_v = psB.tile([AL, 2 * BV], F32, tag="v")
            for i in range(NMT):
                for lo, hi in NSL3:
                    nc.tensor.matmul(ps_v[:, lo:hi], ind8m[i],
                                     big[i][:, lo:hi],
                                     start=(i == 0), stop=(i == NMT - 1))

            # the first t2v group-reduce fills the DVE while the PE runs
            # the indicator matmul (the E3 weights are only needed at the
            # very end)
            nc.vector.reduce_sum(red[0],
                                 big[0].rearrange("p (g v) -> p g v", v=V),
                                 axis=X)

            # vps2 path at [8, x], half-split so DVE/ACT/PE pipeline
            fe4 = bigp.tile([AL, BV], BF16)
            for lo, hi in HALF:
                rdv = smalls.tile([AL, 384], F32, name=f"rdv{lo}")
                nc.vector.reciprocal_approx_fast(
                    rdv, ps_v[:AL, BV + lo:BV + hi])
                v2t = smalls.tile([AL, 384], F32, name=f"v2t{lo}")
                nc.vector.tensor_tensor(v2t, ps_v[:AL, lo:hi], rdv, op=MUL)
                nc.scalar.activation(fe4[:, lo:hi], v2t, EXP, scale=TAU)

            # ---- broadcast E4 over t-rows (PE), evacuate to bf16 (ACT),
            # weight by the bf16 S-hat shadow (DVE 2x), group-sum ----
            ps_w = [psA.tile([128, BV], F32, tag="s", name=f"ps_w{i}")
                    for i in range(NMT)]
            hun = [smalls.tile([128, B], F32, name=f"hun{i}")
                   for i in range(NMT)]
            for i in range(NMT):
                for lo, hi in WSL:
                    nc.tensor.matmul(ps_w[i][:, lo:hi],
                                     cbW[:, 128 * i:128 * (i + 1)],
                                     fe4[:, lo:hi], start=True, stop=True)

            def w4s_stage(i):
                w4b = sqp.tile([128, BV], BF16, name=f"w4b{i}")
                nc.scalar.activation(w4b, ps_w[i], COPY)
                w4s = sqp.tile([128, BV], BF16, name=f"w4s{i}")
                nc.vector.tensor_tensor(w4s, w4b, spb[i], op=MUL)
                nc.vector.reduce_sum(hun[i],
                                     w4s.rearrange("p (g v) -> p g v", v=V),
                                     axis=X)

            def e3_stage(i):
                rdn = smalls.tile([128, B], F32, name=f"rdn{i}")
                nc.vector.reciprocal_approx_fast(rdn, red[i][:, B:])
                t2v = smalls.tile([128, B], F32, name=f"t2v{i}")
                nc.vector.tensor_tensor(t2v, red[i][:, :B], rdn, op=MUL)
                nc.scalar.activation(rhs_f[i][:, B:], t2v, EXP, scale=TAU)

            w4s_stage(0)
            nc.vector.reduce_sum(red[1],
                                 big[1].rearrange("p (g v) -> p g v", v=V),
                                 axis=X)
            e3_stage(0)
            w4s_stage(1)
            e3_stage(1)
            for i in range(NMT):
                nc.vector.tensor_tensor(rhs_f[i][:, :B], rhs_f[i][:, B:],
                                        hun[i], op=MUL)

            ps_o = psB.tile([AL, 128], F32, tag="j")
            for i in range(NMT):
                nc.tensor.matmul(ps_o, ind8[i], rhs_f[i],
                                 start=(i == 0), stop=(i == NMT - 1))
            d4 = smalls.tile([AL, B], F32)
            nc.vector.reduce_sum(d4, fe4.rearrange("p (g v) -> p g v", v=V),
                                 axis=X)
            dd = smalls.tile([AL, B], F32)
            nc.vector.tensor_tensor(dd, ps_o[:AL, B:], d4, op=MUL)
            rdd = smalls.tile([AL, B], F32)
            nc.vector.reciprocal_approx_fast(rdd, dd)
            outw = smalls.tile([AL, B], F32)
            nc.vector.tensor_tensor(outw, ps_o[:AL, :B], rdd, op=MUL)
            nc.sync.dma_start(out=out_d[:, :], in_=outw[:, :])

    nc.compile()
    return nc


_NC_CACHE = None


def _get_program():
    global _NC_CACHE
    if _NC_CACHE is None:
        _NC_CACHE = _build_program()
    return _NC_CACHE


def _make_in_maps(text_feat, video_feat, text_mask):
    bf = ml_dtypes.bfloat16
    vTT = np.ascontiguousarray(video_feat.reshape(BV, D).T).astype(bf)
    vT01 = np.concatenate([vTT[0:128], vTT[128:256]], axis=1)
    vT23 = np.concatenate([vTT[256:384], vTT[384:512]], axis=1)
    # cbW slice i: [8, 128] with cbW[4i + p//T, 128i + p] = 1, so the
    # broadcast matmul copies E4 row 4i+p//T into partition p.
    cbW = np.zeros((AL, 2 * 128), bf)
    for i in range(NMT):
        for p in range(128):
            cbW[APB * i + p // T, 128 * i + p] = 1.0
    in_maps = []
    for c in range(NCORES):
        tsl = text_feat[c * AL:(c + 1) * AL].reshape(AT, D)
        tTT = np.ascontiguousarray(tsl.T).astype(bf)     # [512, 256]
        tTa = np.concatenate([tTT[128 * k:128 * (k + 1)] for k in range(4)],
                             axis=1)                     # [128, 1024]
        mask = text_mask[c * AL:(c + 1) * AL].reshape(AT).astype(np.float32)
        # cb16: [ind8m | ind8 | ones]; indicator col 4i + p//T per M-tile,
        # with the text mask folded into ind8m host-side
        cb16 = np.zeros((128, 33), bf)
        for i in range(NMT):
            for p in range(128):
                col = APB * i + p // T
                cb16[p, 8 * i + col] = mask[128 * i + p]
                cb16[p, 16 + 8 * i + col] = 1.0
        cb16[:, 32] = 1.0
        cb32 = np.zeros((128, 3), np.float32)
        for i in range(NMT):
            cb32[:, i] = TAU * mask[128 * i:128 * (i + 1)]
        cb32[0, 2] = 1.0
        in_maps.append({
            "vT01": vT01,
            "vT23": vT23,
            "tTa": tTa,
            "cb16": cb16,
            "cbW": cbW,
            "cb32": cb32,
        })
    return in_maps


def kernel(text_feat, video_feat, text_mask, _trace=False):
    text_feat = np.asarray(text_feat, dtype=np.float32)
    video_feat = np.asarray(video_feat, dtype=np.float32)
    text_mask = np.asarray(text_mask)
    nc = _get_program()
    in_maps = _make_in_maps(text_feat, video_feat, text_mask)
    res = run_bass_kernel_spmd(nc, in_maps, core_ids=list(range(NCORES)),
                               trace=_trace)
    out = np.concatenate([res.results[c]["out"] for c in range(NCORES)], axis=0)
    if _trace:
        kernel.last_exec_time_ns = res.exec_time_ns
        kernel.last_results = res
    return out


# revision 15
# speedup vs baseline: 1.0215x; 1.0215x over previous
"""Trainium2 Bass kernel for the DCM sparse-attention problem.

Math restructure: with t-hat/v-hat the row-normalized features and
S[(a,t),(b,v)] = <t-hat[a,t], v-hat[b,v]> the raw cosine logits, every
softmax-weighted aggregation in the reference collapses onto S:

  t2v[a,b,t] = sum_v vps1 * S            (free-dim group reduce)
  v2t[a,b,v] = sum_t tps1 * S            (mask-folded indicator matmul)
  out[a,b]   = sum_t tps2[t] sum_v vps2[v] S[t,v]

so the [A,B,T,D] intermediates never exist. Each of the 8 cores handles
8 of the 64 text rows (A-sharded, video replicated).

Dataflow (v5): one tiny merged constant DMA goes first (the indicator
matmuls' ones-column must not queue behind megabyte transfers), then
text, then video in k-granular chunks so squares/norm-matmuls/S-matmuls
stream behind the DMA.  The critical chain is video -> squares ->
column-sum matmul -> sqrt -> reciprocal -> partition-broadcast ->
S-hat; it runs in 384-wide halves to pipeline ACT/DVE/GpSimd.  One
fused scalar_tensor_tensor folds both norms: S-hat = (psum*r_t)*rv,
kept fp32 (bf16 S-hat costs ~20% extra error); a bf16 shadow of S-hat
feeds the later weighting TT in the DVE 2x mode.  Activations are
ordered SQRT then EXP-only (the ACT engine reloads its 1.3us table on
every function switch).  A few dependency-spaced dummy matmuls keep
the PE's HAM clock-gate warm across its idle window.  Indicator
matmuls emit an 8-row layout so the final divides and the single
output DMA stay minimal.
"""

import sys

sys.path.insert(0, "/opt/trn_rl_repo")

import ml_dtypes
import numpy as np

import concourse.bass as bass
import concourse.bacc as bacc
import concourse.tile as tile
from concourse import mybir
from concourse.bass_utils import run_bass_kernel_spmd

TAU = 100.0
A, T, B, V, D = 64, 32, 64, 12, 512
NCORES = 8
AL = A // NCORES          # a's per core = 8
AT = AL * T               # (a,t) rows per core = 256
BV = B * V                # (b,v) cols = 768
NMT = AT // 128           # M-tiles over (a,t) = 2
NKT = D // 128            # K-tiles over d = 4
APB = 128 // T            # a's per M-tile = 4
F32 = mybir.dt.float32
BF16 = mybir.dt.bfloat16
EXP = mybir.ActivationFunctionType.Exp
SQRT = mybir.ActivationFunctionType.Sqrt
COPY = mybir.ActivationFunctionType.Copy
MUL = mybir.AluOpType.mult
X = mybir.AxisListType.X
NSL = [(0, 512), (512, 768)]                   # bank-aligned slices of 768
NSL3 = [(0, 512), (512, 1024), (1024, 1536)]   # ... of 1536
HALF = [(0, 384), (384, 768)]                  # group-aligned halves
WSL = [(0, 384), (384, 512), (512, 768)]       # bank-safe W4 chunks


def _build_program():
    nc = bacc.Bacc("TRN2", target_bir_lowering=False)

    vq_d = [nc.declare_dram_parameter(f"vq{k}", [128, BV], BF16,
                                      isOutput=False) for k in range(NKT)]
    tTa_d = nc.declare_dram_parameter("tTa", [128, 4 * AT], BF16,
                                      isOutput=False)
    # cb: [ind8m (16) | ind8 (16) | ones (1) | m0 (1) | m1 (1) |
    #      cbW broadcast indicator (256, rows 0:8)]
    cb_d = nc.declare_dram_parameter("cb", [128, 291], BF16, isOutput=False)
    out_d = nc.declare_dram_parameter("out", [AL, B], F32, isOutput=True)

    with tile.TileContext(nc) as tc:
        with (
            tc.tile_pool(name="consts", bufs=1) as consts,
            tc.tile_pool(name="inputs", bufs=1) as inputs,
            tc.tile_pool(name="sq", bufs=1) as sqp,
            tc.tile_pool(name="big", bufs=1) as bigp,
            tc.tile_pool(name="smalls", bufs=1) as smalls,
            tc.tile_pool(name="psA", bufs=2, space="PSUM") as psA,
            tc.tile_pool(name="psB", bufs=1, space="PSUM") as psB,
        ):
            # ---- input DMAs, all on the sync queue: constants first
            # (tiny), then text, then video k-chunks ----
            cb = consts.tile([128, 291], BF16)
            vq = [inputs.tile([128, BV], BF16, name=f"vq{k}")
                  for k in range(NKT)]
            tTa = inputs.tile([128, 1024], BF16)
            nc.sync.dma_start(out=cb, in_=cb_d[:, :])
            nc.sync.dma_start(out=vq[0], in_=vq_d[0][:, :])
            nc.sync.dma_start(out=tTa, in_=tTa_d[:, :])
            for k in range(1, NKT):
                nc.sync.dma_start(out=vq[k], in_=vq_d[k][:, :])
            ind8m = [cb[:, 8 * i:8 * (i + 1)] for i in range(NMT)]
            ind8 = [cb[:, 16 + 8 * i:16 + 8 * (i + 1)] for i in range(NMT)]
            onesc = cb[:, 32:33]
            cbW = cb[0:AL, 35:291]
            ident = smalls.tile([1, 1], F32)
            nc.vector.memset(ident, 1.0)
            tau_m = [smalls.tile([128, 1], F32, name=f"tau_m{i}")
                     for i in range(NMT)]
            for i in range(NMT):
                nc.vector.tensor_scalar_mul(tau_m[i], cb[:, 33 + i:34 + i],
                                            TAU)

            def tt(k, i):       # stationary slice for M-tile i, k-tile k
                return tTa[:, 256 * k + 128 * i:256 * k + 128 * (i + 1)]

            # ---- norm + S matmuls interleaved per k-chunk so the PE
            # streams right behind the DMA; squares on the DVE (bf16 2x),
            # ones-matmul column sums on the PE ----
            ps_ssv = psB.tile([1, BV], F32, tag="v")
            ps_s = [psA.tile([128, BV], F32, tag="s", name=f"ps_s{i}")
                    for i in range(NMT)]
            for k in range(NKT):
                sqv = sqp.tile([128, BV], BF16, tag="sqv", name=f"sqv{k}")
                nc.vector.tensor_tensor(sqv, vq[k], vq[k], op=MUL)
                for lo, hi in NSL:
                    nc.tensor.matmul(ps_ssv[:, lo:hi], onesc, sqv[:, lo:hi],
                                     start=(k == 0), stop=(k == NKT - 1))
                for i in range(NMT):
                    for lo, hi in NSL:
                        nc.tensor.matmul(
                            ps_s[i][:, lo:hi], tt(k, i), vq[k][:, lo:hi],
                            start=(k == 0), stop=(k == NKT - 1))
            ps_sst = psB.tile([1, AT], F32, tag="j")
            sqt = sqp.tile([128, 1024], BF16)
            nc.vector.tensor_tensor(sqt, tTa, tTa, op=MUL)
            for k in range(NKT):
                nc.tensor.matmul(ps_sst, onesc, sqt[:, 256 * k:256 * (k + 1)],
                                 start=(k == 0), stop=(k == NKT - 1))

            # both SQRTs back-to-back, then only EXP from here on: the ACT
            # engine reloads its table on EVERY function switch (~1.3us)
            nt_row = smalls.tile([1, AT], F32)
            nc.scalar.activation(nt_row, ps_sst, SQRT)
            nv_row = smalls.tile([1, BV], F32)
            for lo, hi in HALF:
                nc.scalar.activation(nv_row[:, lo:hi], ps_ssv[:, lo:hi], SQRT)

            r_t = [smalls.tile([128, 1], F32, name=f"r_t{i}")
                   for i in range(NMT)]
            for i in range(NMT):
                ps_tr = psB.tile([128, 1], F32, tag="j", name=f"ps_tr{i}")
                nc.tensor.transpose(ps_tr, nt_row[:, 128 * i:128 * (i + 1)],
                                    ident)
                nc.vector.reciprocal_approx_fast(r_t[i], ps_tr)

            # r_v: reciprocal + partition broadcast, in halves so the DVE
            # and GpSimd pipeline
            rv_row = smalls.tile([1, BV], F32)
            rv_bc = bigp.tile([128, BV], F32)
            for lo, hi in HALF:
                nc.vector.reciprocal_approx_fast(rv_row[:, lo:hi],
                                                 nv_row[:, lo:hi])
                nc.gpsimd.partition_broadcast(rv_bc[:, lo:hi],
                                              rv_row[:, lo:hi], channels=128)

            # ---- per-M-tile softmax prep in halves: one fused STT folds
            # both norms into the PSUM read (S-hat = psum*r_t*rv, fp32),
            # the EXP rides the ACT with the tau*m per-partition scale, and
            # ES = S-hat * E closes the pair.  spb is a bf16 shadow of
            # S-hat so the later w4s weighting runs in the DVE 2x mode. ----
            big = [bigp.tile([128, 2 * BV], BF16, name=f"big{i}")
                   for i in range(NMT)]
            sp = [bigp.tile([128, BV], F32, name=f"sp{i}") for i in range(NMT)]
            spb = [bigp.tile([128, BV], BF16, name=f"spb{i}")
                   for i in range(NMT)]
            rhs_f = [smalls.tile([128, 128], BF16, name=f"rhs_f{i}")
                     for i in range(NMT)]
            red = [smalls.tile([128, 128], F32, name=f"red{i}")
                   for i in range(NMT)]
            ps_j = psB.tile([1, 128], F32, tag="j")
            for i in range(NMT):
                for lo, hi in HALF:
                    nc.vector.scalar_tensor_tensor(
                        sp[i][:, lo:hi], ps_s[i][:, lo:hi], r_t[i],
                        rv_bc[:, lo:hi], op0=MUL, op1=MUL)
                    nc.scalar.activation(big[i][:, BV + lo:BV + hi],
                                         sp[i][:, lo:hi], EXP,
                                         scale=tau_m[i][:, :])
                    nc.vector.tensor_tensor(big[i][:, lo:hi], sp[i][:, lo:hi],
                                            big[i][:, BV + lo:BV + hi],
                                            op=MUL)
                    # dependency-spaced dummy matmul keeps the PE's HAM
                    # clock-gate warm through this DVE/ACT-only stretch
                    nc.tensor.matmul(ps_j, onesc,
                                     big[i][:, lo:lo + 128],
                                     start=True, stop=True)
                nc.scalar.activation(spb[i], sp[i], COPY)

            # ---- v2t: mask-folded indicator matmul over t -> [8, 2*BV] ----
            ps_v = psB.tile([AL, 2 * BV], F32, tag="v")
            for i in range(NMT):
                for lo, hi in NSL3:
                    nc.tensor.matmul(ps_v[:, lo:hi], ind8m[i],
                                     big[i][:, lo:hi],
                                     start=(i == 0), stop=(i == NMT - 1))

            # the first t2v group-reduce fills the DVE while the PE runs
            # the indicator matmul (the E3 weights are only needed at the
            # very end)
            nc.vector.reduce_sum(red[0],
                                 big[0].rearrange("p (g v) -> p g v", v=V),
                                 axis=X)

            # vps2 path at [8, x], half-split so DVE/ACT/PE pipeline
            fe4 = bigp.tile([AL, BV], BF16)
            for lo, hi in HALF:
                rdv = smalls.tile([AL, 384], F32, name=f"rdv{lo}")
                nc.vector.reciprocal_approx_fast(
                    rdv, ps_v[:AL, BV + lo:BV + hi])
                v2t = smalls.tile([AL, 384], F32, name=f"v2t{lo}")
                nc.vector.tensor_tensor(v2t, ps_v[:AL, lo:hi], rdv, op=MUL)
                nc.scalar.activation(fe4[:, lo:hi], v2t, EXP, scale=TAU)

            # ---- broadcast E4 over t-rows (PE), evacuate to bf16 (ACT),
            # weight by the bf16 S-hat shadow (DVE 2x), group-sum ----
            ps_w = [psA.tile([128, BV], F32, tag="s", name=f"ps_w{i}")
                    for i in range(NMT)]
            hun = [smalls.tile([128, B], F32, name=f"hun{i}")
                   for i in range(NMT)]
            for i in range(NMT):
                for lo, hi in WSL:
                    nc.tensor.matmul(ps_w[i][:, lo:hi],
                                     cbW[:, 128 * i:128 * (i + 1)],
                                     fe4[:, lo:hi], start=True, stop=True)

            def w4s_stage(i):
                w4b = sqp.tile([128, BV], BF16, name=f"w4b{i}")
                nc.scalar.activation(w4b, ps_w[i], COPY)
                w4s = sqp.tile([128, BV], BF16, name=f"w4s{i}")
                nc.vector.tensor_tensor(w4s, w4b, spb[i], op=MUL)
                nc.vector.reduce_sum(hun[i],
                                     w4s.rearrange("p (g v) -> p g v", v=V),
                                     axis=X)

            def e3_stage(i):
                rdn = smalls.tile([128, B], F32, name=f"rdn{i}")
                nc.vector.reciprocal_approx_fast(rdn, red[i][:, B:])
                t2v = smalls.tile([128, B], F32, name=f"t2v{i}")
                nc.vector.tensor_tensor(t2v, red[i][:, :B], rdn, op=MUL)
                nc.scalar.activation(rhs_f[i][:, B:], t2v, EXP, scale=TAU)

            w4s_stage(0)
            nc.vector.reduce_sum(red[1],
                                 big[1].rearrange("p (g v) -> p g v", v=V),
                                 axis=X)
            e3_stage(0)
            w4s_stage(1)
            e3_stage(1)
            for i in range(NMT):
                nc.vector.tensor_tensor(rhs_f[i][:, :B], rhs_f[i][:, B:],
                                        hun[i], op=MUL)

            ps_o = psB.tile([AL, 128], F32, tag="j")
            for i in range(NMT):
                nc.tensor.matmul(ps_o, ind8[i], rhs_f[i],
                                 start=(i == 0), stop=(i == NMT - 1))
            d4 = smalls.tile([AL, B], F32)
            nc.vector.reduce_sum(d4, fe4.rearrange("p (g v) -> p g v", v=V),
                                 axis=X)
            dd = smalls.tile([AL, B], F32)
            nc.vector.tensor_tensor(dd, ps_o[:AL, B:], d4, op=MUL)
            rdd = smalls.tile([AL, B], F32)
            nc.vector.reciprocal_approx_fast(rdd, dd)
            outw = smalls.tile([AL, B], F32)
            nc.vector.tensor_tensor(outw, ps_o[:AL, :B], rdd, op=MUL)
            nc.sync.dma_start(out=out_d[:, :], in_=outw[:, :])

    nc.compile()
    return nc


_NC_CACHE = None


def _get_program():
    global _NC_CACHE
    if _NC_CACHE is None:
        _NC_CACHE = _build_program()
    return _NC_CACHE


def _make_in_maps(text_feat, video_feat, text_mask):
    bf = ml_dtypes.bfloat16
    vTT = np.ascontiguousarray(video_feat.reshape(BV, D).T).astype(bf)
    in_maps = []
    for c in range(NCORES):
        tsl = text_feat[c * AL:(c + 1) * AL].reshape(AT, D)
        tTT = np.ascontiguousarray(tsl.T).astype(bf)     # [512, 256]
        tTa = np.concatenate([tTT[128 * k:128 * (k + 1)] for k in range(4)],
                             axis=1)                     # [128, 1024]
        mask = text_mask[c * AL:(c + 1) * AL].reshape(AT).astype(np.float32)
        # cb: [ind8m | ind8 | ones | m0 | m1 | cbW]; indicator col
        # 4i + p//T per M-tile, with the text mask folded into ind8m
        # host-side; cbW[4i + p//T, 128i + p] = 1 so the broadcast matmul
        # copies E4 row 4i+p//T into partition p.
        cb = np.zeros((128, 291), bf)
        for i in range(NMT):
            for p in range(128):
                col = APB * i + p // T
                cb[p, 8 * i + col] = mask[128 * i + p]
                cb[p, 16 + 8 * i + col] = 1.0
                cb[col, 35 + 128 * i + p] = 1.0
            cb[:, 33 + i] = mask[128 * i:128 * (i + 1)]
        cb[:, 32] = 1.0
        d = {f"vq{k}": np.ascontiguousarray(vTT[128 * k:128 * (k + 1)])
             for k in range(NKT)}
        d["tTa"] = tTa
        d["cb"] = cb
        in_maps.append(d)
    return in_maps


def kernel(text_feat, video_feat, text_mask, _trace=False):
    text_feat = np.asarray(text_feat, dtype=np.float32)
    video_feat = np.asarray(video_feat, dtype=np.float32)
    text_mask = np.asarray(text_mask)
    nc = _get_program()
    in_maps = _make_in_maps(text_feat, video_feat, text_mask)
    res = run_bass_kernel_spmd(nc, in_maps, core_ids=list(range(NCORES)),
                               trace=_trace)
    out = np.concatenate([res.results[c]["out"] for c in range(NCORES)], axis=0)
    if _trace:
        kernel.last_exec_time_ns = res.exec_time_ns
        kernel.last_results = res
    return out
